# revision 21
# baseline (speedup 1.0000x reference)
"""Trainium2 Bass kernel for MinimalDifferentiableTensorSketch.

Math: out[d] = sum_i w_i * emb[seq_i, d], with w_i = tanh(sign_weight[seq_i])
for valid seq_i in [0, 4), else 0.  Since the alphabet has only 4 valid
symbols (plus the masked value 4), the whole reduction collapses to a
histogram:  out = sum_c count_c * tanh(sign_weight[c]) * emb[c, :].

Device kernel (per core, 2M int32 tokens):
  - DMA 1MB tiles into SBUF (memory-bound stream, ~358 GB/s/core).
  - ACT casts int32 -> bf16 and, fused into the same pass, accumulates
    S1 = sum(x) per partition (activation Copy with accum_out).
  - DVE runs 3 fused compare+reduce passes (is_equal c for c=1,2,3 with
    accum_out), 4x perf mode on bf16.
  - Host recovers c4 = (S1 - c1 - 2c2 - 3c3)/4, c0 = N - c1..c4 (all
    exact integer arithmetic in f64), then combines with the 4x16 table.

Sharding: data-parallel over the sequence axis across 8 cores; the
16-float partials are all-reduced on the host (exact).
"""

import os
import numpy as np

import concourse.bass as bass
from concourse import bacc, mybir
import concourse.tile as tile
from concourse.bass_utils import run_bass_kernel_spmd

N_CORES = 8
SEQ_LEN = 16_777_216
ALPHABET = 4
SKETCH_DIM = 16
P = 128
PER_CORE = SEQ_LEN // N_CORES          # 2,097,152
FREE = PER_CORE // P                   # 16,384 int32 per partition

_NC_CACHE = {}


def _build_nc(
    tile_f=1024,
    fuse_s1=True,
    classes=(1, 2, 3),
    reduce_on_dve=False,
    out_dma_engine="sync",
    tail_split=0,
    pool_cast_stride=0,
    split_out=0,
    free_cols=FREE,
):
    """Build the per-core Bass module.

    Per-tile acc columns (ncols = 1 + max(len(classes), 4)):
      ACT-cast tile:    [S1, cnt(classes[0]), cnt(classes[1]), ...]
      GPSIMD-cast tile: [0,  cnt(1), cnt(2), cnt(3), cnt(4)]  (no S1 fusion)
    pool_cast_stride=k routes every k-th tile's cast to GPSIMD (0 = never).
    """
    n_tiles = free_cols // tile_f
    tiles = [tile_f] * n_tiles
    for _ in range(tail_split):
        last = tiles.pop()
        tiles += [last // 2, last // 2]
    n_tiles = len(tiles)

    pool_classes = (1, 2, 3, 4)
    ncols = 1 + max(len(classes), len(pool_classes) if pool_cast_stride else 0)

    nc = bacc.Bacc(trn_type="TRN2", name="sketch_hist")
    seq = nc.dram_tensor("seq", [P, free_cols], mybir.dt.int32, kind="ExternalInput")
    out_cols = ALPHABET if reduce_on_dve else n_tiles * ncols
    out = nc.dram_tensor(
        "partials", [P, out_cols], mybir.dt.float32, kind="ExternalOutput"
    )
    tile_kinds = []
    with tile.TileContext(nc) as tc:
        with (
            tc.tile_pool(name="accp", bufs=1) as accp,
            tc.tile_pool(name="iop", bufs=n_tiles) as iop,
            tc.tile_pool(name="castp", bufs=n_tiles) as castp,
            tc.tile_pool(name="scrp", bufs=4) as scrp,
        ):
            acc = accp.tile([P, n_tiles, ncols], mybir.dt.float32)
            off = 0
            for t, tf in enumerate(tiles):
                raw = iop.tile([P, tf], mybir.dt.int32, tag="raw")
                nc.sync.dma_start(raw[:], seq[:, off : off + tf])
                off += tf
                b = castp.tile([P, tf], mybir.dt.bfloat16, tag="b")
                use_pool = pool_cast_stride and (t % pool_cast_stride == pool_cast_stride - 1)
                if use_pool:
                    tile_kinds.append("pool")
                    nc.gpsimd.tensor_copy(b[:], raw[:])
                    cls = pool_classes
                elif fuse_s1:
                    tile_kinds.append("act")
                    nc.scalar.activation(
                        b[:],
                        raw[:],
                        mybir.ActivationFunctionType.Copy,
                        accum_out=acc[:, t, 0:1],
                    )
                    cls = classes
                else:
                    tile_kinds.append("plain")
                    nc.scalar.copy(b[:], raw[:])
                    cls = classes
                for k, c in enumerate(cls):
                    scr = scrp.tile([P, tf], mybir.dt.bfloat16, tag="scr")
                    nc.vector.tensor_scalar(
                        scr[:],
                        b[:],
                        float(c),
                        0.0,
                        mybir.AluOpType.is_equal,
                        mybir.AluOpType.add,
                        accum_out=acc[:, t, 1 + k : 2 + k],
                    )
                if split_out and t == split_out - 1:
                    # bulk out-DMA fires once the first split_out tiles'
                    # accums are done, overlapping the tail tiles' compute;
                    # only the last tiles' columns ride the final DMA.
                    nc.sync.dma_start(
                        out[:, : split_out * ncols],
                        acc[:, :split_out, :].rearrange("p t c -> p (t c)"),
                    )
            eng = getattr(nc, out_dma_engine)
            if reduce_on_dve:
                red = accp.tile([P, ALPHABET], mybir.dt.float32)
                acc3 = acc[:].rearrange("p t c -> p c t")
                nc.vector.tensor_reduce(
                    red[:], acc3, mybir.AxisListType.X, mybir.AluOpType.add
                )
                eng.dma_start(out[:], red[:])
            elif split_out:
                eng.dma_start(
                    out[:, split_out * ncols :],
                    acc[:, split_out:, :].rearrange("p t c -> p (t c)"),
                )
            else:
                eng.dma_start(out[:], acc[:].rearrange("p t c -> p (t c)"))
    nc.compile()
    nc._sketch_meta = dict(
        tiles=tiles, fuse_s1=fuse_s1, classes=classes, pool_classes=pool_classes,
        reduce_on_dve=reduce_on_dve, ncols=ncols, tile_kinds=tile_kinds,
    )
    return nc


def _postprocess(results, hash_embedding, sign_weight, meta):
    tiles = meta["tiles"]
    ncols = meta["ncols"]
    n_tiles = len(tiles)
    counts = np.zeros(5, dtype=np.float64)
    if meta["reduce_on_dve"]:
        for r in results:
            counts[:ALPHABET] += r["partials"].astype(np.float64).sum(axis=0)
        counts[0] = SEQ_LEN - counts[1:5].sum() if 0 not in meta["classes"] else counts[0]
    else:
        # exact integer arithmetic in f64 throughout
        s1_act = 0.0           # sum(x) over ACT-cast tiles
        cnt_act = np.zeros(5)  # counts of `classes` over ACT-cast tiles
        for r in results:
            part = r["partials"].astype(np.float64).reshape(P, n_tiles, ncols)
            colsum = part.sum(axis=0)  # [n_tiles, ncols]
            for t, kind in enumerate(meta["tile_kinds"]):
                if kind == "pool":
                    for k, c in enumerate(meta["pool_classes"]):
                        counts[c] += colsum[t, 1 + k]
                elif kind == "act":
                    s1_act += colsum[t, 0]
                    for k, c in enumerate(meta["classes"]):
                        cnt_act[c] += colsum[t, 1 + k]
                else:
                    for k, c in enumerate(meta["classes"]):
                        counts[c] += colsum[t, 1 + k]
        # For ACT tiles: S1 = c1 + 2 c2 + 3 c3 + 4 c4 -> recover c4
        c4_act = (s1_act - cnt_act[1] - 2 * cnt_act[2] - 3 * cnt_act[3]) / 4.0
        counts[1] += cnt_act[1]
        counts[2] += cnt_act[2]
        counts[3] += cnt_act[3]
        counts[4] += c4_act
        counts[0] = SEQ_LEN - counts[1:5].sum()

    w = np.tanh(sign_weight.astype(np.float64))          # [4]
    table = w[:, None] * hash_embedding.astype(np.float64)  # [4, 16]
    out = (counts[:ALPHABET, None] * table).sum(axis=0)
    return out.astype(np.float32)


def kernel(sequence, hash_embedding, sign_weight):
    seq = np.asarray(sequence)
    hash_embedding = np.asarray(hash_embedding, dtype=np.float32)
    sign_weight = np.asarray(sign_weight, dtype=np.float32)

    # int64 input is processed natively: viewed as int32 pairs it becomes
    # [value, 0, value, 0, ...] (values are 0..4, non-negative), and the
    # interleaved zeros only inflate count(0), which the device never
    # counts — c0 is derived on host as N - sum(c1..c4).  The device then
    # streams the full 128 MB as-is (dtype preserved, 2x the columns).
    if seq.dtype == np.int64:
        seq = seq.view(np.int32)
        free_cols = 2 * FREE
    else:
        if seq.dtype != np.int32:
            seq = seq.astype(np.int32)
        free_cols = FREE

    key = ("nc", free_cols)
    if key not in _NC_CACHE:
        _NC_CACHE[key] = _build_nc(
            tile_f=512,
            pool_cast_stride=3,
            split_out=(free_cols // 512) - 2,
            free_cols=free_cols,
        )
    nc = _NC_CACHE[key]
    _NC_CACHE["nc"] = nc  # for test.py's timing fallback

    shards = seq.reshape(N_CORES, P, free_cols)
    in_maps = [{"seq": np.ascontiguousarray(shards[i])} for i in range(N_CORES)]
    res = run_bass_kernel_spmd(
        nc,
        in_maps,
        core_ids=list(range(N_CORES)),
        trace=bool(int(os.environ.get("SKETCH_TRACE", "0"))),
    )
    if res.exec_time_ns is not None:
        print(f"HW exec time: {res.exec_time_ns} ns")
        _NC_CACHE["exec_time_ns"] = res.exec_time_ns
        _NC_CACHE["trace"] = res.instructions_and_trace

    return _postprocess(res.results, hash_embedding, sign_weight, nc._sketch_meta)
